# revision 39
# baseline (speedup 1.0000x reference)
"""Talking-heads attention Trainium2 kernel (Bass/Tile), 8-core data-parallel.

Problem: nn_Attention_talking_head — B=64, N=245, C=768, H=12, D=64,
RPE table (12, 1698) indexed by rel_idx (245, 245), talking-heads mixing
(12x12) before and after softmax, in/out projections.

Sharding: batch 64 -> 8 cores x 8 batches. Weights replicated. No collectives.

v2 design (all-bf16 PE pipeline, instruction-count-minimized):
  - All matmul operands bf16 (1 cyc/row regardless of free size; f32 PSUM
    accumulation). fp32 only at x load, exp input (PSUM), sums, final y.
  - Softmax without max-subtraction (logits bounded, exp safe in f32).
  - RPE bias written into the premix PSUM bank via a PE matmul with identity
    stationary (start=True); the block-diag premix accumulates on top
    (start=False). ACT exp then reads pre-biased PSUM directly, evicting
    P (bf16) + row-sums (accum_out) in ONE instruction per j.
  - Post-softmax normalization folded into the postmix moving matrix:
    bd2j = bd2_pattern / rowsum (one DVE tensor_scalar divide per j).
    Postmix (lhsT = P) fuses the transpose back to [m, packed] layout.
  - Packed layout row = nb*12 + h, n = 31*nb + j (NGRP=8 groups fixed by
    the 16-partition ap_gather core grouping); N padded to 248 = 8*31 so
    the (nb, j) split is exact.
  - PSUM evictions batched multi-tile-per-bank; spread over ACT/DVE/gpsimd.
"""
import os
import numpy as np
from contextlib import ExitStack

import concourse.bass as bass
import concourse.tile as tile
from concourse import bacc, mybir, library_config
from concourse.bass_utils import run_bass_kernel_spmd
from concourse.masks import make_identity

F32 = mybir.dt.float32
BF16 = mybir.dt.bfloat16
I32 = mybir.dt.int32
I16 = mybir.dt.int16
EXP = mybir.ActivationFunctionType.Exp
IDENT = mybir.ActivationFunctionType.Identity
ADD = mybir.AluOpType.add
AX = mybir.AxisListType.X
MULT = mybir.AluOpType.mult
DIV = mybir.AluOpType.divide

NCORES = 8
B, N, C, H, D = 64, 245, 768, 12, 64
BLOC = B // NCORES          # 8 batches per core
E = 3 * C                   # 2304
NBKT = 1698
SCALE = D ** -0.5
NGRP = 8                    # packed n-groups (fixed by 16-partition gather cores)
NJ = 31                     # packed tiles per batch; n = 31*nb + j
NP = NGRP * H               # 96 packed rows
NPAD = NGRP * NJ            # 248 padded n (exact (nb, j) split)
NIDX = 7600                 # gather stream length per group (31*245 real + 5 pad)
CC = C // 128               # 6 contraction chunks
MCS = [(0, 128), (128, 117)]  # (m offset, size) chunks of 245



def _ecopy(nc, eng, out, in_):
    if eng is nc.scalar:
        nc.scalar.copy(out=out, in_=in_)
    else:
        eng.tensor_copy(out=out, in_=in_)

def _emit(ctx: ExitStack, tc, io):
    nc = tc.nc
    x_d, wqkv_d, wproj_d, bproj_d, wl_d, ww_d, bw_d, pbias_d, out_d = io

    const = ctx.enter_context(tc.tile_pool(name="const", bufs=1))
    ctx0 = ctx.enter_context(ExitStack())
    tmp = ctx0.enter_context(tc.tile_pool(name="tmp", bufs=1))
    ps_su = ctx0.enter_context(tc.tile_pool(name="ps_su", bufs=2, space="PSUM"))

    identf = const.tile([128, 128], F32)
    make_identity(nc, identf[:])
    identb = const.tile([128, 128], BF16)
    make_identity(nc, identb[:])

    # ---- weight transposes (PE, f32 in -> bf16 out); single-DMA loads ----
    wqkv_sb = tmp.tile([128, 18, C], F32, tag="wqL")
    for wch in range(3):
        nc.sync.dma_start(
            out=wqkv_sb[:, wch * 6:(wch + 1) * 6, :],
            in_=wqkv_d.rearrange("(e p) c -> p e c", p=128)[:, wch * 6:(wch + 1) * 6])
    wproj_sb = tmp.tile([128, CC, C], F32, tag="wpL")
    nc.sync.dma_start(out=wproj_sb[:],
                      in_=wproj_d.rearrange("(e p) c -> p e c", p=128))
    wqkvT = const.tile([128, CC, E], BF16)   # [c-part, c-chunk, e]
    for ec in range(E // 128):
        for cp in range(3):
            pst = ps_su.tile([128, 2, 128], F32, tag="su")
            for ci in range(2):
                cc = cp * 2 + ci
                nc.tensor.transpose(out=pst[:, ci, :],
                                    in_=wqkv_sb[:, ec, cc * 128:(cc + 1) * 128],
                                    identity=identf[:])
            eng = (nc.scalar, nc.vector)[(ec + cp) % 2]
            _ecopy(nc, eng, wqkvT[:, cp * 2:cp * 2 + 2, ec * 128:(ec + 1) * 128],
                   pst[:])
    wprojT = const.tile([128, CC, C], BF16)
    for ec in range(CC):
        for cp in range(3):
            pst = ps_su.tile([128, 2, 128], F32, tag="su")
            for ci in range(2):
                cc = cp * 2 + ci
                nc.tensor.transpose(out=pst[:, ci, :],
                                    in_=wproj_sb[:, ec, cc * 128:(cc + 1) * 128],
                                    identity=identf[:])
            eng = (nc.scalar, nc.vector)[(ec + cp) % 2]
            _ecopy(nc, eng, wprojT[:, cp * 2:cp * 2 + 2, ec * 128:(ec + 1) * 128],
                   pst[:])

    # ---- w_l / w_w transposes; block-diagonal mixers ----
    wl_sb = tmp.tile([12, 12], F32, tag="wsml")
    nc.sync.dma_start(out=wl_sb[:], in_=wl_d[:, :])
    ps12 = ps_su.tile([12, 12], F32, tag="sml")
    nc.tensor.transpose(out=ps12[:], in_=wl_sb[:], identity=identf[:12, :12])
    wlT_scaled = tmp.tile([12, 12], F32, tag="wsml3")
    nc.scalar.mul(out=wlT_scaled[:], in_=ps12[:], mul=SCALE)

    ww_sb = tmp.tile([12, 12], F32, tag="wsml")
    nc.sync.dma_start(out=ww_sb[:], in_=ww_d[:, :])
    ps12b = ps_su.tile([12, 12], F32, tag="sml")
    nc.tensor.transpose(out=ps12b[:], in_=ww_sb[:], identity=identf[:12, :12])
    wwT = tmp.tile([12, 12], F32, tag="wsml4")
    nc.scalar.copy(out=wwT[:], in_=ps12b[:])

    # nb-major packing: row p = nb*12 + h -> contiguous 12x12 diagonal blocks.
    # Engine writes must start at 32-aligned partitions, so assemble in f32
    # scratch via DMA block copies, then cast with one aligned copy.
    bd1_f32 = tmp.tile([NP, NP], F32, tag="bd1f")
    nc.vector.memset(bd1_f32[:], 0.0)
    bd2_f32 = tmp.tile([NP, NP], F32, tag="bd2f")
    nc.vector.memset(bd2_f32[:], 0.0)
    for nb in range(NGRP):
        s = nb * H
        nc.gpsimd.dma_start(out=bd1_f32[s:s + H, s:s + H], in_=wlT_scaled[:])
        nc.gpsimd.dma_start(out=bd2_f32[s:s + H, s:s + H], in_=wwT[:])
    bd1 = const.tile([NP, NP], BF16)   # [(nb,h), (nb,g)] = SCALE*w_l[g,h]
    nc.scalar.copy(out=bd1[:], in_=bd1_f32[:])
    bd2p = const.tile([NP, NP], BF16)  # [(nb,h), (nb,g)] = w_w[g,h]
    nc.vector.tensor_copy(out=bd2p[:], in_=bd2_f32[:])

    # ---- packed raw RPE bias / SCALE (host-folded; premix matmul applies
    # the w_l mix and SCALE, so adding this to pk before premix yields
    # premix(S) + mixed-bias) ----
    packed_bias = const.tile([NP, NJ * N], BF16)
    nc.sync.dma_start(out=packed_bias[:], in_=pbias_d[:, :])

    # ---- small constants ----
    bw_exp = const.tile([128, CC], F32)   # bw_exp[p, t] = b_w[2t + p//64]
    for half in range(2):
        nc.gpsimd.dma_start(
            out=bw_exp[half * 64:(half + 1) * 64, :],
            in_=bw_d[half:12:2].unsqueeze(0).to_broadcast([64, CC]))
    bproj_sb = const.tile([128, C], F32)
    nc.gpsimd.dma_start(out=bproj_sb[:], in_=bproj_d[:].unsqueeze(0).to_broadcast([128, C]))
    onesb = const.tile([128, 1], BF16)
    nc.vector.memset(onesb[:], 1.0)

    ctx0.close()

    # ---- per-batch streaming pools ----
    xb_p = ctx.enter_context(tc.tile_pool(name="xb", bufs=2))
    xT_p = ctx.enter_context(tc.tile_pool(name="xT", bufs=2))
    qT_p = ctx.enter_context(tc.tile_pool(name="qT", bufs=2))
    kT_p = ctx.enter_context(tc.tile_pool(name="kT", bufs=2))
    v_p = ctx.enter_context(tc.tile_pool(name="v", bufs=3))
    sq_p = ctx.enter_context(tc.tile_pool(name="sq", bufs=3))
    pk_p = ctx.enter_context(tc.tile_pool(name="pk", bufs=2))
    pj_p = ctx.enter_context(tc.tile_pool(name="pj", bufs=5))
    b2_p = ctx.enter_context(tc.tile_pool(name="b2", bufs=6))
    at_p = ctx.enter_context(tc.tile_pool(name="at", bufs=2))
    oT_p = ctx.enter_context(tc.tile_pool(name="oT", bufs=2))
    y_p = ctx.enter_context(tc.tile_pool(name="y", bufs=2))
    st_p = ctx.enter_context(tc.tile_pool(name="st", bufs=8))

    ps_a = ctx.enter_context(tc.tile_pool(name="ps_a", bufs=2, space="PSUM"))
    ps_pk = ctx.enter_context(tc.tile_pool(name="ps_pk", bufs=2, space="PSUM"))
    ps_mix = ctx.enter_context(tc.tile_pool(name="ps_mix", bufs=4, space="PSUM"))

    # ================= software-pipelined batch loop =================
    # Stage A(b): x load/transpose, QKV GEMM, bwv, QK^T -> swt.
    # Stage B(b): j-loop (premix/softmax/postmix), AV, projection.
    # Emission interleaves B(b) with A(b+1) so the in-order PE stream always
    # has independent work when B's cross-engine chains stall, keeping the
    # PE busy (and clocked up). PSUM pools are disjoint per stage.
    NBATCH = BLOC
    state = {}

    def genA(b):
        xb = xb_p.tile([128, 2, C], F32, name="xb")
        for mc, (mo, msz) in enumerate(MCS):
            nc.sync.dma_start(out=xb[:msz, mc, :], in_=x_d[b, mo:mo + msz, :])
        yield
        xT = xT_p.tile([128, CC, NPAD], BF16, name="xT")
        for cp in range(3):
            psxt = ps_a.tile([128, 4, 128], F32, tag="a", name="psxt")
            for ci in range(2):
                cc = cp * 2 + ci
                for mc, (mo, msz) in enumerate(MCS):
                    nc.tensor.transpose(out=psxt[:, ci * 2 + mc, :msz],
                                        in_=xb[:msz, mc, cc * 128:(cc + 1) * 128],
                                        identity=identf[:msz, :msz])
            for mc, (mo, msz) in enumerate(MCS):
                src = psxt[:].rearrange("p (ci mc) m -> p ci mc m", mc=2)[:, :, mc, :msz]
                _ecopy(nc, nc.scalar, xT[:, cp * 2:cp * 2 + 2, mo:mo + msz], src)
            yield
        qT = qT_p.tile([128, CC, NPAD], BF16, name="qT")
        nc.vector.memset(qT[:], 0.0)
        kT = kT_p.tile([128, CC, NPAD], BF16, name="kT")
        for pair in range(6):
            psqk = ps_a.tile([128, 2, 256], F32, tag="a", name="psqk")
            for i in range(2):
                ec = pair * 2 + i
                for cc in range(CC):
                    nc.tensor.matmul(out=psqk[:, i, :N],
                                     lhsT=wqkvT[:, cc, ec * 128:(ec + 1) * 128],
                                     rhs=xT[:, cc, :N],
                                     start=(cc == 0), stop=(cc == CC - 1))
            dst = qT if pair < 3 else kT
            dc = (pair % 3) * 2
            eng = (nc.scalar, nc.vector)[pair % 2]
            _ecopy(nc, eng, dst[:, dc:dc + 2, :N], psqk[:, :, :N])
            yield
        v_sb = v_p.tile([128, 2, C], BF16, name="v_sb")
        for vc in range(2):
            for mc, (mo, msz) in enumerate(MCS):
                psv = ps_a.tile([128, 384], F32, tag="a", name="psv")
                for cc in range(CC):
                    nc.tensor.matmul(
                        out=psv[:msz], lhsT=xT[:, cc, mo:mo + msz],
                        rhs=wqkvT[:, cc, 2 * C + vc * 384:2 * C + (vc + 1) * 384],
                        start=(cc == 0), stop=(cc == CC - 1))
                _ecopy(nc, nc.scalar, v_sb[:msz, mc, vc * 384:(vc + 1) * 384], psv[:msz])
                yield
        psbw = ps_a.tile([128, 8], F32, tag="a", name="psbw")
        for t in range(CC):
            for mc, (mo, msz) in enumerate(MCS):
                nc.tensor.matmul(out=psbw[:, t:t + 1],
                                 lhsT=v_sb[:msz, mc, t * 128:(t + 1) * 128],
                                 rhs=onesb[:msz, :],
                                 start=(mc == 0), stop=(mc == 1))
        bwv = st_p.tile([128, CC], F32, tag="bwv", name="bwv")
        nc.vector.tensor_tensor(out=bwv[:], in0=psbw[:, :CC], in1=bw_exp[:], op=MULT)
        yield
        # QK^T -> swt [m, mc, (j, nb, h)] bf16, packed column order.
        # Heads paired same-parity: a 2-slot PSUM bank must see a single
        # stationary partition offset (alternation in-bank crashes the device).
        swt = sq_p.tile([128, 2, NJ * NP], BF16, name="swt")
        HPAIRS = [(0, 2), (4, 6), (8, 10), (1, 3), (5, 7), (9, 11)]
        for mc, (mo, msz) in enumerate(MCS):
            for pi, hp in enumerate(HPAIRS):
                psqq = ps_a.tile([128, 2, NPAD], F32, tag="a", name="psqq")
                for i, g in enumerate(hp):
                    nc.tensor.matmul(
                        out=psqq[:msz, i, :],
                        lhsT=kT[(g % 2) * 64:(g % 2) * 64 + 64, g // 2, mo:mo + msz],
                        rhs=qT[(g % 2) * 64:(g % 2) * 64 + 64, g // 2, :],
                        start=True, stop=True)
                eng = (nc.vector, nc.scalar)[pi % 2]
                dst = swt[:msz, mc].rearrange(
                    "p (j nb h) -> p h nb j", nb=NGRP, h=H)[:, hp[0]:hp[1] + 1:2]
                src = psqq[:msz].rearrange("p g (nb j) -> p g nb j", nb=NGRP)
                _ecopy(nc, eng, dst, src)
                yield
        state[b] = (swt, v_sb, bwv)

    def genB(b0):
        # Fused 2-batch attention: the packed softmax tiles hold both
        # batches side by side in the free dim ([96, (bb, m)]), halving
        # premix/exp/reduce/recip/pk-evict instruction counts.
        bbs = [b0, b0 + 1]
        swt0, v_sb0, bwv0 = state.pop(b0)
        swt1, v_sb1, bwv1 = state.pop(b0 + 1)
        swts, v_sbs, bwvs = [swt0, swt1], [v_sb0, v_sb1], [bwv0, bwv1]
        atw0 = at_p.tile([128, 2, H, NPAD], BF16, name="atw0")
        atw1 = at_p.tile([128, 2, H, NPAD], BF16, name="atw1")
        atws = [atw0, atw1]
        DPIPE = 3
        pjs, b2s, psats = {}, {}, {}

        def produce(j):
            pkps = ps_pk.tile([128, 2, 256], BF16, tag="pk", name="pkps")
            for bb in range(2):
                for mc, (mo, msz) in enumerate(MCS):
                    nc.tensor.transpose(out=pkps[:NP, bb, mo:mo + msz],
                                        in_=swts[bb][:msz, mc, j * NP:(j + 1) * NP],
                                        identity=identb[:msz, :msz])
            pk_sb = pk_p.tile([128, 2, N], BF16, name="pk_sb")
            nc.vector.tensor_tensor(
                out=pk_sb[:NP, :, :N], in0=pkps[:NP, :, :N],
                in1=packed_bias[:, j * N:(j + 1) * N].unsqueeze(1).to_broadcast(
                    [NP, 2, N]),
                op=ADD)
            psm = ps_mix.tile([128, 2, N], F32, tag="mix", name="psm")
            nc.tensor.matmul(out=psm[:NP, :, :].rearrange("p b n -> p (b n)"),
                             lhsT=bd1[:],
                             rhs=pk_sb[:NP, :, :].rearrange("p b n -> p (b n)"),
                             start=True, stop=True)
            pj2 = pj_p.tile([128, 2, NPAD], BF16, tag="pj", name="pj2")
            nc.scalar.activation(out=pj2[:NP, :, :N], in_=psm[:NP, :, :N],
                                 func=EXP, scale=1.0)
            ssum2 = st_p.tile([128, 2], F32, tag="ss", name="ssum2")
            nc.vector.tensor_reduce(out=ssum2[:NP, :], in_=pj2[:NP, :, :N],
                                    axis=AX, op=ADD)
            rec2 = st_p.tile([128, 2], F32, tag="rc", name="rec2")
            nc.vector.reciprocal(out=rec2[:NP, :], in_=ssum2[:NP, :])
            bds = []
            for bb in range(2):
                bd2j = b2_p.tile([128, NP], BF16, tag="b2", name="bd2j")
                nc.vector.tensor_scalar(out=bd2j[:NP, :], in0=bd2p[:],
                                        scalar1=rec2[:NP, bb:bb + 1],
                                        scalar2=None, op0=MULT)
                bds.append(bd2j)
            pjs[j], b2s[j] = pj2, bds

        def consume(j):
            jj = j % 2
            if jj == 0:
                psats[j] = [ps_mix.tile([128, 2, 2, NP], F32, tag="mix", name="psat0k"),
                            ps_mix.tile([128, 2, 2, NP], F32, tag="mix", name="psat1k")]
            psat = psats[j - jj]       # per mc: [jj, bb, NP]
            pj2, bds = pjs.pop(j), b2s.pop(j)
            for bb in range(2):
                for mc, (mo, msz) in enumerate(MCS):
                    nc.tensor.matmul(out=psat[mc][:msz, jj, bb, :],
                                     lhsT=pj2[:NP, bb, mo:mo + msz],
                                     rhs=bds[bb][:NP, :], start=True, stop=True)
            if jj == 1 or j == NJ - 1:
                jp = (j - jj) // 2
                npair = jj + 1
                for bb in range(2):
                    for mc, (mo, msz) in enumerate(MCS):
                        eng = nc.scalar if (bb + mc) % 2 == 0 else nc.vector
                        dst = atws[bb][:msz, mc].rearrange(
                            "p h (nb j) -> p h nb j", j=NJ)[
                                :, :, :, jp * 2:jp * 2 + npair]
                        src = psat[mc][:msz, :npair, bb, :].rearrange(
                            "p jj (nb h) -> p h nb jj", nb=NGRP)
                        _ecopy(nc, eng, dst, src)
                del psats[j - jj]

        for j in range(NJ + DPIPE):
            if j < NJ:
                produce(j)
            if j >= DPIPE:
                consume(j - DPIPE)
            if j % 2 == 1:
                yield

        for bb in range(2):
            outT = oT_p.tile([128, CC, NPAD], BF16, name="outT")
            for g in range(H):
                psav = ps_pk.tile([128, NPAD], F32, tag="pk", name="psav")
                for mc, (mo, msz) in enumerate(MCS):
                    nc.tensor.matmul(out=psav[:64, :],
                                     lhsT=v_sbs[bb][:msz, mc, g * 64:(g + 1) * 64],
                                     rhs=atws[bb][:msz, mc, g, :],
                                     start=(mc == 0), stop=(mc == 1))
                if g % 4 < 2:
                    nc.scalar.activation(
                        out=outT[(g % 2) * 64:(g % 2) * 64 + 64, g // 2, :N],
                        in_=psav[:64, :N], func=IDENT,
                        bias=bwvs[bb][(g % 2) * 64:(g % 2) * 64 + 64,
                                      g // 2:g // 2 + 1],
                        scale=1.0)
                else:
                    nc.vector.tensor_scalar_add(
                        out=outT[(g % 2) * 64:(g % 2) * 64 + 64, g // 2, :N],
                        in0=psav[:64, :N],
                        scalar1=bwvs[bb][(g % 2) * 64:(g % 2) * 64 + 64,
                                         g // 2:g // 2 + 1])
                if g % 2 == 1:
                    yield
            for mc, (mo, msz) in enumerate(MCS):
                y = y_p.tile([128, C], F32, name="y")
                for half in range(2):
                    psy = ps_pk.tile([128, 384], F32, tag="pk", name="psy")
                    for cc in range(CC):
                        nc.tensor.matmul(
                            out=psy[:msz], lhsT=outT[:, cc, mo:mo + msz],
                            rhs=wprojT[:, cc, half * 384:(half + 1) * 384],
                            start=(cc == 0), stop=(cc == CC - 1))
                    nc.vector.tensor_tensor(
                        out=y[:msz, half * 384:(half + 1) * 384], in0=psy[:msz],
                        in1=bproj_sb[:msz, half * 384:(half + 1) * 384], op=ADD)
                nc.sync.dma_start(out=out_d[bbs[bb], mo:mo + msz, :], in_=y[:msz, :])
                yield

    def drain(g):
        if g is None:
            return True
        try:
            next(g)
            return False
        except StopIteration:
            return True

    for _ in genA(0):
        pass
    for _ in genA(1):
        pass
    for bp in range(NBATCH // 2):
        b0 = bp * 2
        gb = genB(b0)
        nxt = [b0 + 2 + i for i in range(2) if b0 + 2 + i < NBATCH]
        gas = [genA(nb) for nb in nxt]
        done_b = False
        ai = 0
        while not done_b or gas:
            if not done_b:
                done_b = drain(gb)
            if gas:
                if drain(gas[0]):
                    gas.pop(0)
